# revision 2
# baseline (speedup 1.0000x reference)
"""Trainium2 Bass kernel for nn_ConceptEmbedding (type-conditioned embedding lookup).

Reference computation (per token position (b, s)):
    t = token_type[b, s]
    out[b, s, :] = proc_emb[concept]  if t == 1
                   med_emb[concept]   if t == 2
                   chart_emb[concept] if t == 3
                   0                  otherwise

Strategy (v4):
  - Fold the three tables into one [3V, E] table with flat row index
    (t-1)*V + concept. Tokens with t outside {1,2,3} produce zeros and are
    never sent to the device (the host assembles their rows as zeros).
  - Shard tokens across the 8 cores BY TABLE ROW RANGE: core c owns rows
    [c*37504, (c+1)*37504). The host hands each core a contiguous slice of
    the table ("twin", 37632 rows) as its per-core input, so all gather
    windows have static bases. ~3072 typed tokens land on each core.
  - Device (per core): the HW gather (InstDMAGatherAnt) takes int16 indices,
    so the 37632-row band is covered by two windows (0:32768 and
    32768:37632). Four dma_gather instructions on the four SWDGE queues pull
    the rows into one SBUF buffer; four stores (two on sync/SP-HWDGE, two on
    scalar/ACT-HWDGE), each gated on its own gather's completion semaphore,
    pipeline the write-back behind the remaining gathers' drains.
  - v4 latency hiding (vs v3's serial chain):
      * idx upload moved to sync/HWDGE so it overlaps the ~9us GPSIMD mlp
        library reload instead of queuing behind it.
      * a 128-idx warm-up dma_gather per SWDGE queue right after the library
        load absorbs the first-use cold cost (v3 saw 8.7us blocking on the
        first real gather) while the idx upload is still in flight.
      * pad slots use index -1 (trailing negatives are skipped by the Q7
        descriptor generator) instead of 0, so padding costs no descriptors
        and no HBM traffic.
      * caps trimmed to observed maxima + margin (2816+512 vs 3072+512).
  - Host: buckets/sorts tokens by row (ascending - also gives the DMA
    ascending HBM addresses), pads buckets to the fixed caps, unpermutes the
    result while assembling the full [B, S, E] output. Bucket overflow
    beyond the caps (statistical tail) is gathered on host.

dma_gather layout facts (verified on HW):
  - indices live at idxs[i % 16, i // 16], int16, replicated across all 128
    partitions; valid index i lands at dst[i % 128, i // 128, :].
  - one instruction must stay near ~1024 indices (the SWDGE descriptor ring
    is small; 1792-index gathers crash the exec unit).
  - the store view maps SBUF (p, block b) -> DRAM row p*NB + b, so the DRAM
    row for slot j of a window at block base B0 is (j % 128)*NB + B0 + j//128.
"""

import numpy as np

V = 100000
E = 128
B = 16
S = 2048
NCORES = 8
P = 128

N_TOK = B * S  # 32768
NROWS = 3 * V  # 300000

RSPAN = 37504  # table rows owned per core (8 * 37504 >= 300000)
TWLEN = 37632  # per-core table slice length (RSPAN + 128 alignment margin)
W0 = 32768  # window 0 covers twin[0:32768]
W1LEN = TWLEN - W0  # 4864 rows in window 1

# Slot caps. Window 0 expects ~2690 typed tokens per core (observed max 2750),
# window 1 ~390 (max 435); host gathers the statistical-tail overflow.
W0CAP = 2816
W1CAP = 512
SUMCAP = W0CAP + W1CAP  # 3328
NB = SUMCAP // P  # 26 blocks
W0BLOCKS = W0CAP // P  # 22

# (window, slot cap, swdge queue): one gather per queue; queue q runs on Q7
# core pair (2q, 2q+1) so the four generations proceed concurrently.
GATHERS = [(0, 1024, 0), (0, 1024, 1), (0, 768, 2), (1, 512, 3)]
WARM = 128  # warm-up gather size per queue

_CACHED_NC = None


def _build_bass():
    global _CACHED_NC
    if _CACHED_NC is not None:
        return _CACHED_NC

    import concourse.bacc as bacc
    import concourse.mybir as mybir
    from concourse.library_config import mlp

    # Raw Bacc Block (no Tile): explicit semaphores avoid Tile's multi-engine
    # teardown barrier cascade (~9us) and most of its sem-clear preamble.
    nc = bacc.Bacc(num_swdge_queues=4)
    twin = nc.dram_tensor("twin", [TWLEN, E], mybir.dt.float32, kind="ExternalInput")
    idx = nc.dram_tensor("idx", [P, SUMCAP // 16], mybir.dt.int16, kind="ExternalInput")
    out = nc.dram_tensor("out", [SUMCAP, E], mybir.dt.float32, kind="ExternalOutput")

    # SBUF (p, block b) <-> DRAM row p*NB + b
    out_v = out.rearrange("(p b) e -> p (b e)", p=P)

    with (
        nc.Block() as block,
        nc.sbuf_tensor("dst", [P, NB * E], mybir.dt.float32) as dst,
        nc.sbuf_tensor("idxs", [P, SUMCAP // 16], mybir.dt.int16) as idxs,
        nc.sbuf_tensor("widx", [P, WARM // 16], mybir.dt.int16) as widx,
        nc.sbuf_tensor("wdst", [P, 4 * E], mybir.dt.float32) as wdst,
        nc.semaphore("io") as io,
        nc.semaphore("msem") as msem,
        nc.semaphore("wsem") as wsem,
        nc.semaphore("s0") as s0,
        nc.semaphore("s1") as s1,
        nc.semaphore("s2") as s2,
        nc.semaphore("s3") as s3,
    ):
        ssems = [s0, s1, s2, s3]
        # Block boundaries: gather k fills dst blocks [bases[k], bases[k+1]).
        bases = [0]
        for _, cap, _ in GATHERS:
            bases.append(bases[-1] + cap // P)

        @block.vector
        def _(vector):
            # Zeroed warm-up indices (a negative garbage index would be an
            # OOB HBM read; index 0 just re-reads twin row 0).
            vector.memset(widx[:], 0).then_inc(msem, 1)

        @block.gpsimd
        def _(gpsimd):
            gpsimd.load_library(mlp)
            gpsimd.wait_ge(msem, 1)
            for q in range(4):
                d3 = wdst[:, q * E : (q + 1) * E].rearrange("p (b e) -> p b e", e=E)
                gpsimd.dma_gather(
                    d3, twin[0:W0, :], widx[:, :], WARM, WARM, E, queue_num=q
                ).then_inc(wsem, 16)
            gpsimd.wait_ge(io, 16)
            off16 = 0
            for k, (w, cap, qn) in enumerate(GATHERS):
                in_ap = twin[0:W0, :] if w == 0 else twin[W0:TWLEN, :]
                d3 = dst[:, bases[k] * E : bases[k + 1] * E].rearrange(
                    "p (b e) -> p b e", e=E
                )
                gpsimd.dma_gather(
                    d3,
                    in_ap,
                    idxs[:, off16 : off16 + cap // 16],
                    cap,
                    cap,
                    E,
                    queue_num=qn,
                ).then_inc(ssems[k], 16)
                off16 += cap // 16

        @block.sync
        def _(sync):
            sync.dma_start(out=idxs[:], in_=idx[:]).then_inc(io, 16)
            for k in (0, 2):
                sync.wait_ge(ssems[k], 16)
                sync.dma_start(
                    out=out_v[:, bases[k] * E : bases[k + 1] * E],
                    in_=dst[:, bases[k] * E : bases[k + 1] * E],
                ).then_inc(io, 16)
            sync.wait_ge(wsem, 16 * 4)
            sync.wait_ge(io, 16 * 5)

        @block.scalar
        def _(scalar):
            for k in (1, 3):
                scalar.wait_ge(ssems[k], 16)
                scalar.dma_start(
                    out=out_v[:, bases[k] * E : bases[k + 1] * E],
                    in_=dst[:, bases[k] * E : bases[k + 1] * E],
                ).then_inc(io, 16)

    nc.finalize()
    _CACHED_NC = nc
    return nc


def _shard_inputs(proc_emb, med_emb, chart_emb, concept, token_type):
    """Returns (in_maps, plans, tables) with per-core slot bookkeeping."""
    tables = np.ascontiguousarray(
        np.concatenate(
            [
                np.asarray(proc_emb, dtype=np.float32),
                np.asarray(med_emb, dtype=np.float32),
                np.asarray(chart_emb, dtype=np.float32),
            ],
            axis=0,
        )
    )
    tt = np.asarray(token_type).reshape(-1).astype(np.int64)
    cc = np.asarray(concept).reshape(-1).astype(np.int64)
    typed = (tt >= 1) & (tt <= 3)
    toks_all = np.where(typed)[0]  # global token ids with a real lookup
    eff = cc[toks_all] + (tt[toks_all] - 1) * V  # their table rows

    core_of = eff // RSPAN
    local = eff - core_of * RSPAN

    in_maps = []
    plans = []  # per core: (tokens, dram_rows, overflow_tokens, overflow_rows)
    for c in range(NCORES):
        base = c * RSPAN
        sl = tables[base : min(base + TWLEN, NROWS)]
        if sl.shape[0] < TWLEN:
            sl = np.concatenate([sl, np.zeros((TWLEN - sl.shape[0], E), np.float32)])
        twin = np.ascontiguousarray(sl)

        sel = np.where(core_of == c)[0]
        order = sel[np.argsort(local[sel], kind="stable")]
        lrows = local[order]  # ascending
        n0 = int(np.searchsorted(lrows, W0))  # tokens in window 0
        win_lists = [
            (lrows[:n0], toks_all[order[:n0]], W0CAP, 0, 0),
            (lrows[n0:] - W0, toks_all[order[n0:]], W1CAP, W0CAP, W0BLOCKS),
        ]

        idx16 = np.zeros((16, SUMCAP // 16), dtype=np.int16)
        tok_list, row_list, ovf_toks, ovf_rows = [], [], [], []
        for wrows, wtoks, cap, slot0, b0 in win_lists:
            cnt = len(wrows)
            if cnt > cap:
                # Statistical-tail safety valve: gather the overflow on host.
                ovf_toks.extend(wtoks[cap:].tolist())
                ovf_rows.extend((wrows[cap:] + (0 if slot0 == 0 else W0)).tolist())
                wrows, wtoks, cnt = wrows[:cap], wtoks[:cap], cap
            vals = np.full(cap, -1, dtype=np.int16)  # trailing -1 pads are skipped
            vals[:cnt] = wrows.astype(np.int16)
            idx16[:, slot0 // 16 : (slot0 + cap) // 16] = vals.reshape(cap // 16, 16).T
            j = np.arange(cnt)
            row_list.append((j % P) * NB + b0 + j // P)
            tok_list.append(wtoks)

        in_maps.append(
            {"twin": twin, "idx": np.ascontiguousarray(np.tile(idx16, (8, 1)))}
        )
        plans.append(
            (
                np.concatenate(tok_list),
                np.concatenate(row_list),
                np.array(ovf_toks, dtype=np.int64),
                np.array(ovf_rows, dtype=np.int64) + base,
            )
        )

    return in_maps, plans, tables


def _run(in_maps, trace=False):
    from concourse.bass_utils import run_bass_kernel_spmd

    nc = _build_bass()
    return run_bass_kernel_spmd(nc, in_maps, list(range(NCORES)), trace=trace)


def _assemble(results, plans, tables):
    out = np.zeros((N_TOK, E), dtype=np.float32)
    for c in range(NCORES):
        toks, drows, ovf_toks, ovf_rows = plans[c]
        if len(toks):
            out[toks] = results[c]["out"][drows]
        if len(ovf_toks):
            out[ovf_toks] = tables[ovf_rows]
    return out.reshape(B, S, E)


def kernel(proc_emb, med_emb, chart_emb, concept, token_type):
    in_maps, plans, tables = _shard_inputs(
        proc_emb, med_emb, chart_emb, concept, token_type
    )
    res = _run(in_maps, trace=False)
    return _assemble(res.results, plans, tables)


# revision 3
# speedup vs baseline: 1.0304x; 1.0304x over previous
"""Trainium2 Bass kernel for nn_ConceptEmbedding (type-conditioned embedding lookup).

Reference computation (per token position (b, s)):
    t = token_type[b, s]
    out[b, s, :] = proc_emb[concept]  if t == 1
                   med_emb[concept]   if t == 2
                   chart_emb[concept] if t == 3
                   0                  otherwise

Strategy (v5):
  - Fold the three tables into one [3V, E] table with flat row index
    (t-1)*V + concept. Tokens with t outside {1,2,3} produce zeros and are
    never sent to the device (the host assembles their rows as zeros).
  - Shard tokens across the 8 cores BY TABLE ROW RANGE: core c owns rows
    [c*37504, (c+1)*37504). The host hands each core a contiguous slice of
    the table ("twin", 37632 rows) as its per-core input, so all gather
    windows have static bases. ~3072 typed tokens land on each core.
  - Device (per core): the HW gather (InstDMAGatherAnt) takes int16 indices,
    so the 37632-row band is covered by two windows (0:32768 and
    32768:37632). Four dma_gather instructions pull the rows into one SBUF
    buffer; four stores (two on sync/SP-HWDGE, two on scalar/ACT-HWDGE),
    each gated on its own gather's completion semaphore, pipeline the
    write-back behind the remaining drains.
  - SWDGE queue facts measured on HW (v3/v4 traces):
      * queue 0 runs descriptor generation INLINE on the GpSimd engine
        (~8.5 ns/idx blocking); queues 1-3 hand off to their Q7 core pair
        and the issue returns in ~60 ns.
      * the FIRST dma_gather after the mlp library reload pays a cold-start
        (it ran 8.7us for 1024 idx in v3); a 128-idx warm-up gather absorbs
        it in ~1.6us, overlapped shadow work.
      * a second gather on the same queue busy-waits for descriptor-ring
        space until the first one drains (4us seen in v3).
    Hence: one 128-idx warm-up on q1, the three window-0 gathers async on
    q1/q2/q3, and the small window-1 gather inline on q0 issued LAST - its
    ~4us inline generation overlaps the other queues' HBM drains.
  - idx upload runs on sync/HWDGE so it overlaps the ~9us GPSIMD mlp
    library reload. Warm-up indices are host-staged (128 distinct rows -
    all-zero indices hammer one HBM row, serializing the warm-up drain).
  - Pad slots use index -1 (trailing negatives are skipped by the Q7
    descriptor generator): padding costs no descriptors and no HBM traffic.
  - Host: buckets/sorts tokens by row (ascending - also gives the DMA
    ascending HBM addresses), pads buckets to the fixed caps, unpermutes the
    result while assembling the full [B, S, E] output. Bucket overflow
    beyond the caps (statistical tail) is gathered on host.

dma_gather layout facts (verified on HW):
  - indices live at idxs[i % 16, i // 16], int16, replicated across all 128
    partitions; valid index i lands at dst[i % 128, i // 128, :].
  - one instruction must stay near ~1024 indices (the SWDGE descriptor ring
    is small; 1792-index gathers crash the exec unit).
  - the store view maps SBUF (p, block b) -> DRAM row p*NB + b, so the DRAM
    row for slot j of a window at block base B0 is (j % 128)*NB + B0 + j//128.
"""

import numpy as np

V = 100000
E = 128
B = 16
S = 2048
NCORES = 8
P = 128

N_TOK = B * S  # 32768
NROWS = 3 * V  # 300000

RSPAN = 37504  # table rows owned per core (8 * 37504 >= 300000)
TWLEN = 37632  # per-core table slice length (RSPAN + 128 alignment margin)
W0 = 32768  # window 0 covers twin[0:32768]
W1LEN = TWLEN - W0  # 4864 rows in window 1

# Slot caps. Window 0 expects ~2690 typed tokens per core (observed max 2750),
# window 1 ~390 (max 435); host gathers the statistical-tail overflow.
W0CAP = 2816
W1CAP = 512
SUMCAP = W0CAP + W1CAP  # 3328
NB = SUMCAP // P  # 26 blocks
W0BLOCKS = W0CAP // P  # 22

# Device issue order: (window, slot0, cap, queue). Window-0 slots 0:2816 are
# split 1024+1024+768 across async queues 1-3; window-1 slots 2816:3328 go
# inline on q0, issued last.
GATHERS = [
    (0, 0, 1024, 1),
    (0, 1024, 1024, 2),
    (0, 2048, 768, 3),
    (1, 2816, 512, 0),
]
WARM = 128  # warm-up gather size
IDXCOLS = SUMCAP // 16 + WARM // 16  # 208 data + 8 warm-up columns

_CACHED_NC = None


def _build_bass():
    global _CACHED_NC
    if _CACHED_NC is not None:
        return _CACHED_NC

    import concourse.bacc as bacc
    import concourse.mybir as mybir
    from concourse.library_config import mlp

    # Raw Bacc Block (no Tile): explicit semaphores avoid Tile's multi-engine
    # teardown barrier cascade (~9us) and most of its sem-clear preamble.
    nc = bacc.Bacc(num_swdge_queues=4)
    twin = nc.dram_tensor("twin", [TWLEN, E], mybir.dt.float32, kind="ExternalInput")
    idx = nc.dram_tensor("idx", [P, IDXCOLS], mybir.dt.int16, kind="ExternalInput")
    out = nc.dram_tensor("out", [SUMCAP, E], mybir.dt.float32, kind="ExternalOutput")

    # SBUF (p, block b) <-> DRAM row p*NB + b
    out_v = out.rearrange("(p b) e -> p (b e)", p=P)

    with (
        nc.Block(no_gpsimd_drain=True) as block,
        nc.sbuf_tensor("dst", [P, NB * E], mybir.dt.float32) as dst,
        nc.sbuf_tensor("idxs", [P, IDXCOLS], mybir.dt.int16) as idxs,
        nc.sbuf_tensor("wdst", [P, E], mybir.dt.float32) as wdst,
        nc.semaphore("io") as io,
        nc.semaphore("wsem") as wsem,
        nc.semaphore("s0") as s0,
        nc.semaphore("s1") as s1,
        nc.semaphore("s2") as s2,
        nc.semaphore("s3") as s3,
    ):
        ssems = [s0, s1, s2, s3]

        @block.gpsimd
        def _(gpsimd):
            gpsimd.load_library(mlp)
            gpsimd.wait_ge(io, 16)
            wd3 = wdst[:, :].rearrange("p (b e) -> p b e", e=E)
            gpsimd.dma_gather(
                wd3,
                twin[0:W0, :],
                idxs[:, SUMCAP // 16 : IDXCOLS],
                WARM,
                WARM,
                E,
                queue_num=1,
            ).then_inc(wsem, 16)
            for k, (w, slot0, cap, qn) in enumerate(GATHERS):
                in_ap = twin[0:W0, :] if w == 0 else twin[W0:TWLEN, :]
                b0 = slot0 // P
                d3 = dst[:, b0 * E : (b0 + cap // P) * E].rearrange(
                    "p (b e) -> p b e", e=E
                )
                gpsimd.dma_gather(
                    d3,
                    in_ap,
                    idxs[:, slot0 // 16 : (slot0 + cap) // 16],
                    cap,
                    cap,
                    E,
                    queue_num=qn,
                ).then_inc(ssems[k], 16)

        @block.sync
        def _(sync):
            sync.dma_start(out=idxs[:], in_=idx[:]).then_inc(io, 16)
            for k in (0, 1):
                _, slot0, cap, _ = GATHERS[k]
                b0, b1 = slot0 // P, (slot0 + cap) // P
                sync.wait_ge(ssems[k], 16)
                sync.dma_start(
                    out=out_v[:, b0 * E : b1 * E], in_=dst[:, b0 * E : b1 * E]
                ).then_inc(io, 16)
            sync.wait_ge(wsem, 16)
            sync.wait_ge(io, 16 * 5)

        @block.scalar
        def _(scalar):
            for k in (2, 3):
                _, slot0, cap, _ = GATHERS[k]
                b0, b1 = slot0 // P, (slot0 + cap) // P
                scalar.wait_ge(ssems[k], 16)
                scalar.dma_start(
                    out=out_v[:, b0 * E : b1 * E], in_=dst[:, b0 * E : b1 * E]
                ).then_inc(io, 16)

    nc.finalize()
    _CACHED_NC = nc
    return nc


def _shard_inputs(proc_emb, med_emb, chart_emb, concept, token_type):
    """Returns (in_maps, plans, tables) with per-core slot bookkeeping."""
    tables = np.ascontiguousarray(
        np.concatenate(
            [
                np.asarray(proc_emb, dtype=np.float32),
                np.asarray(med_emb, dtype=np.float32),
                np.asarray(chart_emb, dtype=np.float32),
            ],
            axis=0,
        )
    )
    tt = np.asarray(token_type).reshape(-1).astype(np.int64)
    cc = np.asarray(concept).reshape(-1).astype(np.int64)
    typed = (tt >= 1) & (tt <= 3)
    toks_all = np.where(typed)[0]  # global token ids with a real lookup
    eff = cc[toks_all] + (tt[toks_all] - 1) * V  # their table rows

    core_of = eff // RSPAN
    local = eff - core_of * RSPAN

    # Warm-up indices: 128 distinct, page-spread rows of window 0.
    warm16 = (np.arange(WARM, dtype=np.int16) * 256).reshape(WARM // 16, 16).T

    in_maps = []
    plans = []  # per core: (tokens, dram_rows, overflow_tokens, overflow_rows)
    for c in range(NCORES):
        base = c * RSPAN
        sl = tables[base : min(base + TWLEN, NROWS)]
        if sl.shape[0] < TWLEN:
            sl = np.concatenate([sl, np.zeros((TWLEN - sl.shape[0], E), np.float32)])
        twin = np.ascontiguousarray(sl)

        sel = np.where(core_of == c)[0]
        order = sel[np.argsort(local[sel], kind="stable")]
        lrows = local[order]  # ascending
        n0 = int(np.searchsorted(lrows, W0))  # tokens in window 0
        win_lists = [
            (lrows[:n0], toks_all[order[:n0]], W0CAP, 0, 0),
            (lrows[n0:] - W0, toks_all[order[n0:]], W1CAP, W0CAP, W0BLOCKS),
        ]

        idx16 = np.zeros((16, IDXCOLS), dtype=np.int16)
        idx16[:, SUMCAP // 16 :] = warm16
        tok_list, row_list, ovf_toks, ovf_rows = [], [], [], []
        for wrows, wtoks, cap, slot0, b0 in win_lists:
            cnt = len(wrows)
            if cnt > cap:
                # Statistical-tail safety valve: gather the overflow on host.
                ovf_toks.extend(wtoks[cap:].tolist())
                ovf_rows.extend((wrows[cap:] + (0 if slot0 == 0 else W0)).tolist())
                wrows, wtoks, cnt = wrows[:cap], wtoks[:cap], cap
            vals = np.full(cap, -1, dtype=np.int16)  # trailing -1 pads are skipped
            vals[:cnt] = wrows.astype(np.int16)
            idx16[:, slot0 // 16 : (slot0 + cap) // 16] = vals.reshape(cap // 16, 16).T
            j = np.arange(cnt)
            row_list.append((j % P) * NB + b0 + j // P)
            tok_list.append(wtoks)

        in_maps.append(
            {"twin": twin, "idx": np.ascontiguousarray(np.tile(idx16, (8, 1)))}
        )
        plans.append(
            (
                np.concatenate(tok_list),
                np.concatenate(row_list),
                np.array(ovf_toks, dtype=np.int64),
                np.array(ovf_rows, dtype=np.int64) + base,
            )
        )

    return in_maps, plans, tables


def _run(in_maps, trace=False):
    from concourse.bass_utils import run_bass_kernel_spmd

    nc = _build_bass()
    return run_bass_kernel_spmd(nc, in_maps, list(range(NCORES)), trace=trace)


def _assemble(results, plans, tables):
    out = np.zeros((N_TOK, E), dtype=np.float32)
    for c in range(NCORES):
        toks, drows, ovf_toks, ovf_rows = plans[c]
        if len(toks):
            out[toks] = results[c]["out"][drows]
        if len(ovf_toks):
            out[ovf_toks] = tables[ovf_rows]
    return out.reshape(B, S, E)


def kernel(proc_emb, med_emb, chart_emb, concept, token_type):
    in_maps, plans, tables = _shard_inputs(
        proc_emb, med_emb, chart_emb, concept, token_type
    )
    res = _run(in_maps, trace=False)
    return _assemble(res.results, plans, tables)
